# revision 21
# baseline (speedup 1.0000x reference)
"""ApproxSiLU16_FXP Trainium2 kernel (8 NeuronCores, data-parallel).

The reference computes a 16-segment piecewise-linear fixed-point
approximation of SiLU on a uniform knot grid t_k = -8 + 0.875k
(k = 0..16), with knot values y_k = round(1024*silu(t_k))/1024.
Instead of gathering from the LUT per element, this kernel
reconstructs the same piecewise-linear function analytically:

    u   = x*(8/7) + 64/7          (segment coordinate, in [0,16])
    k   = floor(u)
    fr  = u - k
    out = silu(t_k) + fr*(silu(t_k+0.875) - silu(t_k))

using the ScalarEngine's Silu activation for the knot values.  This
matches the fixed-point reference to ~2e-3 relative error (reference
LUT int rounding + fp16 rounding of the blend), well under the 2e-2
gate.

floor() uses a small-magic fp16 trick: u' = u - 0.5 rounded to fp16,
then kfm = fp16(u' + 1029).  1029 + [-0.5, 16.1] lies inside the fp16
binade [1024, 2048) where the fp16 ulp is exactly 1.0, so the output
rounding (DVE computes fp32 internally, rounds on the write) snaps to
1029 + floor(u).  Both Silus then use the free input FMA:
t_k = 0.875*kfm - 908.375 (exact in fp32).  fr = u' - (kfm - 1029.5)
needs one more fp16 scalar-add and one fp16 subtract, all in fast
16-bit DVE modes.

Engine split per tile (balanced ~183us each at FD=2048):
  ACT : u' (Copy FMA -> fp16, most tiles), a = silu(...), b = silu(...)
  DVE : kfm, kfm2 (fp16 1-scalar add, ~4x), fr, t, g, o (fp16 2x TT)
out = a - (a-b)*fr, stored fp16; the host upcasts to fp32.

Sharding: x is (8, 2048, 4096); core i processes batch row i.
"""

import numpy as np

from concourse import bacc, mybir
import concourse.tile as tile
from concourse.bass_utils import run_bass_kernel_spmd

F32 = mybir.dt.float32
F16 = mybir.dt.float16
Alu = mybir.AluOpType
Act = mybir.ActivationFunctionType

P = 128          # SBUF partitions
FD = 2048        # free dim per tile
NT = 32          # tiles per core shard: 2048*4096 = NT*P*FD
N_CORES = 8

C87 = float(8.0 / 7.0)
UP_BIAS = float(64.0 / 7.0 - 0.5)
MAGIC = 1029.0           # fp16 binade [1024,2048): ulp = 1
MAGIC2 = -1029.5         # kfm2 = k - 0.5
SILU_BIAS_A = -908.375   # 0.875*(k+1029) - 908.375 = 0.875k - 8 = t_k
SILU_BIAS_B = -907.5     # t_k + 0.875


def _reg_const(nc, val):
    t = nc.alloc_sbuf_tensor(f"const-f32-{val}", [128, 1], F32)
    nc.gpsimd.memset(t.ap(), val)
    nc.const_aps.aps[(F32, float(val))] = t.ap()


def build():
    nc = bacc.Bacc()
    _reg_const(nc, SILU_BIAS_A)
    _reg_const(nc, SILU_BIAS_B)
    nc.all_engine_barrier()
    x_ext = nc.declare_dram_parameter("x", [NT, P, FD], F32, isOutput=False)
    o_ext = nc.declare_dram_parameter("out", [NT, P, FD], F16, isOutput=True)

    with tile.TileContext(nc) as tc, tc.tile_pool(name="p", bufs=5) as pool:
        for i in range(NT):
            xt = pool.tile([P, FD], F32, tag="xt", bufs=3)
            nc.sync.dma_start(xt[:], x_ext[i])
            # u' = x*(8/7) + (64/7 - 0.5), fp16 out.  Mostly on ACT; a small
            # share on DVE to balance engine load.
            up = pool.tile([P, FD], F16, tag="up")
            if i % 8 == 7:
                nc.vector.tensor_scalar(
                    up[:], xt[:], C87, UP_BIAS, Alu.mult, Alu.add
                )
            else:
                nc.scalar.activation(up[:], xt[:], Act.Copy, bias=UP_BIAS, scale=C87)
            # kfm = fp16(u' + 1029) = 1029 + floor(u)   (fp16 ulp-1 snap)
            kfm = pool.tile([P, FD], F16, tag="kfm")
            nc.vector.tensor_single_scalar(kfm[:], up[:], MAGIC, Alu.add)
            # kfm2 = k - 0.5   (fp16 exact)
            kfm2 = pool.tile([P, FD], F16, tag="kfm2")
            nc.vector.tensor_single_scalar(kfm2[:], kfm[:], MAGIC2, Alu.add)
            # fr = u' - kfm2 = u - k   (fp16 2x)
            fr = pool.tile([P, FD], F16, tag="fr")
            nc.vector.tensor_tensor(fr[:], up[:], kfm2[:], Alu.subtract)
            # a = silu(t_k), b = silu(t_k + 0.875)   (fp16, free input FMA)
            a = pool.tile([P, FD], F16, tag="a")
            nc.scalar.activation(a[:], kfm[:], Act.Silu, bias=SILU_BIAS_A, scale=0.875)
            b = pool.tile([P, FD], F16, tag="b")
            nc.scalar.activation(b[:], kfm[:], Act.Silu, bias=SILU_BIAS_B, scale=0.875)
            # t = a - b   (fp16 2x)
            t = pool.tile([P, FD], F16, tag="t")
            nc.vector.tensor_tensor(t[:], a[:], b[:], Alu.subtract)
            # g = t*fr = (a-b)*fr   (fp16 2x)
            g = pool.tile([P, FD], F16, tag="g")
            nc.vector.tensor_tensor(g[:], t[:], fr[:], Alu.mult)
            # o = a - g = a + (b-a)*fr   (fp16 2x)
            o = pool.tile([P, FD], F16, tag="o")
            nc.vector.tensor_tensor(o[:], a[:], g[:], Alu.subtract)
            nc.sync.dma_start(o_ext[i], o[:])
    nc.compile()
    return nc


_NC_CACHE = None


def _get_nc():
    global _NC_CACHE
    if _NC_CACHE is None:
        _NC_CACHE = build()
    return _NC_CACHE


def _ensure_ntff_hook():
    """Install the antenv.axon_hooks shim so trace=True works under axon."""
    import sys
    import types

    if "antenv.axon_hooks" not in sys.modules:
        mod = types.ModuleType("antenv.axon_hooks")
        _h = [None]
        mod.set_axon_ntff_profile_hook = lambda h: _h.__setitem__(0, h)
        mod.get_axon_ntff_profile_hook = lambda: _h[0]
        sys.modules["antenv.axon_hooks"] = mod
        import antenv

        antenv.axon_hooks = mod
    import antenv.axon_hooks as ah

    if ah.get_axon_ntff_profile_hook() is None:
        from trn_agent_boot.trn_boot import _ntff_profile_via_ctypes

        h = _ntff_profile_via_ctypes("/opt/axon/libaxon_pjrt.so")
        if h is not None:
            ah.set_axon_ntff_profile_hook(h)
    # avoid cloud artifact uploads in this container
    import concourse.bass_utils as bu

    bu.upload_artifacts = lambda tmpdir: tmpdir


def _run_once(x, trace=False, trace_kwargs=None):
    nc = _get_nc()
    core_ids = list(range(N_CORES))
    in_maps = [{"x": x[i].reshape(NT, P, FD)} for i in range(N_CORES)]
    kwargs = {}
    if trace:
        _ensure_ntff_hook()
        kwargs["trace"] = True
        if trace_kwargs:
            kwargs.update(trace_kwargs)
    res = run_bass_kernel_spmd(nc, in_maps, core_ids, **kwargs)
    out = np.empty((N_CORES, 2048, 4096), dtype=np.float32)
    for i in range(N_CORES):
        out[i] = np.asarray(res.results[i]["out"], dtype=np.float32).reshape(
            2048, 4096
        )
    return out, res.exec_time_ns


def _run(x, trace=False, trace_kwargs=None):
    """x: (8, 2048, 4096) float32. Returns (out, exec_time_ns|None)."""
    x = np.ascontiguousarray(np.asarray(x, dtype=np.float32))
    assert x.shape == (N_CORES, 2048, 4096), x.shape
    # The axon terminal occasionally reports a transient unrecoverable
    # error on the first execution of a freshly loaded NEFF; retry.
    last_exc = None
    for _attempt in range(3):
        try:
            return _run_once(x, trace=trace, trace_kwargs=trace_kwargs)
        except Exception as e:  # noqa: BLE001
            last_exc = e
            import time

            time.sleep(2.0)
    raise last_exc


def kernel(x, seg=None, silu_vals=None, **_unused):
    out, _ = _run(x, trace=False)
    return out


# revision 22
# speedup vs baseline: 1.1806x; 1.1806x over previous
"""ApproxSiLU16_FXP Trainium2 kernel (8 NeuronCores, data-parallel).

The reference computes a 16-segment piecewise-linear fixed-point
approximation of SiLU on a uniform knot grid t_k = -8 + 0.875k
(k = 0..16), with knot values y_k = round(1024*silu(t_k))/1024.
Instead of gathering from the LUT per element, this kernel
reconstructs the same piecewise-linear function analytically:

    u   = x*(8/7) + 64/7          (segment coordinate, in [0,16])
    k   = floor(u)
    fr  = u - k
    out = silu(t_k) + fr*(silu(t_k+0.875) - silu(t_k))

using the ScalarEngine's Silu activation for the knot values.  This
matches the fixed-point reference to ~2e-3 relative error (reference
LUT int rounding + fp16 rounding of the blend), well under the 2e-2
gate.

floor() uses a small-magic fp16 trick: u' = u - 0.5 rounded to fp16,
then kfm = fp16(u' + 1029).  1029 + [-0.5, 16.1] lies inside the fp16
binade [1024, 2048) where the fp16 ulp is exactly 1.0, so the output
rounding (DVE computes fp32 internally, rounds on the write) snaps to
1029 + floor(u).  Both Silus then use the free input FMA:
t_k = 0.875*kfm - 908.375 (exact in fp32).  fr = u' - (kfm - 1029.5)
needs one more fp16 scalar-add and one fp16 subtract, all in fast
16-bit DVE modes.

Engine split per tile (balanced ~183us each at FD=2048):
  ACT : u' (Copy FMA -> fp16, most tiles), a = silu(...), b = silu(...)
  DVE : kfm, kfm2 (fp16 1-scalar add, ~4x), fr, t, g, o (fp16 2x TT)
out = a - (a-b)*fr, stored fp16; the host upcasts to fp32.

Sharding: x is (8, 2048, 4096); core i processes batch row i.
"""

import numpy as np

from concourse import bacc, mybir
import concourse.tile as tile
from concourse.bass_utils import run_bass_kernel_spmd

F32 = mybir.dt.float32
F16 = mybir.dt.float16
Alu = mybir.AluOpType
Act = mybir.ActivationFunctionType

P = 128          # SBUF partitions
FD = 2048        # free dim per tile
NT = 32          # tiles per core shard: 2048*4096 = NT*P*FD
N_CORES = 8

C87 = float(8.0 / 7.0)
UP_BIAS = float(64.0 / 7.0 - 0.5)
MAGIC = 1029.0           # fp16 binade [1024,2048): ulp = 1
MAGIC2 = -1029.5         # kfm2 = k - 0.5
SILU_BIAS_A = -908.375   # 0.875*(k+1029) - 908.375 = 0.875k - 8 = t_k
SILU_BIAS_B = -907.5     # t_k + 0.875


def _reg_const(nc, val):
    t = nc.alloc_sbuf_tensor(f"const-f32-{val}", [128, 1], F32)
    nc.gpsimd.memset(t.ap(), val)
    nc.const_aps.aps[(F32, float(val))] = t.ap()


def build():
    nc = bacc.Bacc()
    _reg_const(nc, SILU_BIAS_A)
    _reg_const(nc, SILU_BIAS_B)
    nc.all_engine_barrier()
    x_ext = nc.declare_dram_parameter("x", [NT, P, FD], F32, isOutput=False)
    o_ext = nc.declare_dram_parameter("out", [NT, P, FD], F16, isOutput=True)

    with tile.TileContext(nc) as tc, tc.tile_pool(name="p", bufs=4) as pool:
        for i in range(NT):
            xt = pool.tile([P, FD], F32, tag="xt")
            nc.sync.dma_start(xt[:], x_ext[i])
            # u' = x*(8/7) + (64/7 - 0.5), fp16 out.  Mostly on ACT; a small
            # share on DVE to balance engine load.
            up = pool.tile([P, FD], F16, tag="up")
            if i % 8 == 7:
                nc.vector.tensor_scalar(
                    up[:], xt[:], C87, UP_BIAS, Alu.mult, Alu.add
                )
            else:
                nc.scalar.activation(up[:], xt[:], Act.Copy, bias=UP_BIAS, scale=C87)
            # kfm = fp16(u' + 1029) = 1029 + floor(u)   (fp16 ulp-1 snap)
            kfm = pool.tile([P, FD], F16, tag="kfm")
            nc.vector.tensor_single_scalar(kfm[:], up[:], MAGIC, Alu.add)
            # kfm2 = k - 0.5   (fp16 exact)
            kfm2 = pool.tile([P, FD], F16, tag="kfm2")
            nc.vector.tensor_single_scalar(kfm2[:], kfm[:], MAGIC2, Alu.add)
            # fr = u' - kfm2 = u - k   (fp16 2x)
            fr = pool.tile([P, FD], F16, tag="fr")
            nc.vector.tensor_tensor(fr[:], up[:], kfm2[:], Alu.subtract)
            # a = silu(t_k), b = silu(t_k + 0.875)   (fp16, free input FMA)
            a = pool.tile([P, FD], F16, tag="a")
            nc.scalar.activation(a[:], kfm[:], Act.Silu, bias=SILU_BIAS_A, scale=0.875)
            b = pool.tile([P, FD], F16, tag="b")
            nc.scalar.activation(b[:], kfm[:], Act.Silu, bias=SILU_BIAS_B, scale=0.875)
            # t = a - b   (fp16 2x)
            t = pool.tile([P, FD], F16, tag="t")
            nc.vector.tensor_tensor(t[:], a[:], b[:], Alu.subtract)
            # g = t*fr = (a-b)*fr   (fp16 2x)
            g = pool.tile([P, FD], F16, tag="g")
            nc.vector.tensor_tensor(g[:], t[:], fr[:], Alu.mult)
            # o = a - g = a + (b-a)*fr   (fp16 2x)
            o = pool.tile([P, FD], F16, tag="o")
            nc.vector.tensor_tensor(o[:], a[:], g[:], Alu.subtract)
            nc.sync.dma_start(o_ext[i], o[:])
    nc.compile()
    return nc


_NC_CACHE = None


def _get_nc():
    global _NC_CACHE
    if _NC_CACHE is None:
        _NC_CACHE = build()
    return _NC_CACHE


def _ensure_ntff_hook():
    """Install the antenv.axon_hooks shim so trace=True works under axon."""
    import sys
    import types

    if "antenv.axon_hooks" not in sys.modules:
        mod = types.ModuleType("antenv.axon_hooks")
        _h = [None]
        mod.set_axon_ntff_profile_hook = lambda h: _h.__setitem__(0, h)
        mod.get_axon_ntff_profile_hook = lambda: _h[0]
        sys.modules["antenv.axon_hooks"] = mod
        import antenv

        antenv.axon_hooks = mod
    import antenv.axon_hooks as ah

    if ah.get_axon_ntff_profile_hook() is None:
        from trn_agent_boot.trn_boot import _ntff_profile_via_ctypes

        h = _ntff_profile_via_ctypes("/opt/axon/libaxon_pjrt.so")
        if h is not None:
            ah.set_axon_ntff_profile_hook(h)
    # avoid cloud artifact uploads in this container
    import concourse.bass_utils as bu

    bu.upload_artifacts = lambda tmpdir: tmpdir


def _run_once(x, trace=False, trace_kwargs=None):
    nc = _get_nc()
    core_ids = list(range(N_CORES))
    in_maps = [{"x": x[i].reshape(NT, P, FD)} for i in range(N_CORES)]
    kwargs = {}
    if trace:
        _ensure_ntff_hook()
        kwargs["trace"] = True
        if trace_kwargs:
            kwargs.update(trace_kwargs)
    res = run_bass_kernel_spmd(nc, in_maps, core_ids, **kwargs)
    out = np.empty((N_CORES, 2048, 4096), dtype=np.float32)
    for i in range(N_CORES):
        out[i] = np.asarray(res.results[i]["out"], dtype=np.float32).reshape(
            2048, 4096
        )
    return out, res.exec_time_ns


def _run(x, trace=False, trace_kwargs=None):
    """x: (8, 2048, 4096) float32. Returns (out, exec_time_ns|None)."""
    x = np.ascontiguousarray(np.asarray(x, dtype=np.float32))
    assert x.shape == (N_CORES, 2048, 4096), x.shape
    # The axon terminal occasionally reports a transient unrecoverable
    # error on the first execution of a freshly loaded NEFF; retry.
    last_exc = None
    for _attempt in range(3):
        try:
            return _run_once(x, trace=trace, trace_kwargs=trace_kwargs)
        except Exception as e:  # noqa: BLE001
            last_exc = e
            import time

            time.sleep(2.0)
    raise last_exc


def kernel(x, seg=None, silu_vals=None, **_unused):
    out, _ = _run(x, trace=False)
    return out


# revision 23
# speedup vs baseline: 1.1928x; 1.0104x over previous
"""ApproxSiLU16_FXP Trainium2 kernel (8 NeuronCores, data-parallel).

The reference computes a 16-segment piecewise-linear fixed-point
approximation of SiLU on a uniform knot grid t_k = -8 + 0.875k
(k = 0..16), with knot values y_k = round(1024*silu(t_k))/1024.
Instead of gathering from the LUT per element, this kernel
reconstructs the same piecewise-linear function analytically:

    u   = x*(8/7) + 64/7          (segment coordinate, in [0,16])
    k   = floor(u)
    fr  = u - k
    out = silu(t_k) + fr*(silu(t_k+0.875) - silu(t_k))

using the ScalarEngine's Silu activation for the knot values.  This
matches the fixed-point reference to ~2e-3 relative error (reference
LUT int rounding + fp16 rounding of the blend), well under the 2e-2
gate.

floor() uses a small-magic fp16 trick: u' = u - 0.5 rounded to fp16,
then kfm = fp16(u' + 1029).  1029 + [-0.5, 16.1] lies inside the fp16
binade [1024, 2048) where the fp16 ulp is exactly 1.0, so the output
rounding (DVE computes fp32 internally, rounds on the write) snaps to
1029 + floor(u).  Both Silus then use the free input FMA:
t_k = 0.875*kfm - 908.375 (exact in fp32).  fr = u' - (kfm - 1029.5)
needs one more fp16 scalar-add and one fp16 subtract, all in fast
16-bit DVE modes.

Engine split per tile (balanced ~183us each at FD=2048):
  ACT : u' (Copy FMA -> fp16, most tiles), a = silu(...), b = silu(...)
  DVE : kfm, kfm2 (fp16 1-scalar add, ~4x), fr, t, g, o (fp16 2x TT)
out = a - (a-b)*fr, stored fp16; the host upcasts to fp32.

Sharding: x is (8, 2048, 4096); core i processes batch row i.
"""

import numpy as np

from concourse import bacc, mybir
import concourse.tile as tile
from concourse.bass_utils import run_bass_kernel_spmd

F32 = mybir.dt.float32
F16 = mybir.dt.float16
Alu = mybir.AluOpType
Act = mybir.ActivationFunctionType

P = 128          # SBUF partitions
FD = 4096        # free dim per tile
NT = 16          # tiles per core shard: 2048*4096 = NT*P*FD
N_CORES = 8

C87 = float(8.0 / 7.0)
UP_BIAS = float(64.0 / 7.0 - 0.5)
MAGIC = 1029.0           # fp16 binade [1024,2048): ulp = 1
MAGIC2 = -1029.5         # kfm2 = k - 0.5
SILU_BIAS_A = -908.375   # 0.875*(k+1029) - 908.375 = 0.875k - 8 = t_k
SILU_BIAS_B = -907.5     # t_k + 0.875


def _reg_const(nc, val):
    t = nc.alloc_sbuf_tensor(f"const-f32-{val}", [128, 1], F32)
    nc.gpsimd.memset(t.ap(), val)
    nc.const_aps.aps[(F32, float(val))] = t.ap()


def build():
    nc = bacc.Bacc()
    _reg_const(nc, SILU_BIAS_A)
    _reg_const(nc, SILU_BIAS_B)
    nc.all_engine_barrier()
    x_ext = nc.declare_dram_parameter("x", [NT, P, FD], F32, isOutput=False)
    o_ext = nc.declare_dram_parameter("out", [NT, P, FD], F16, isOutput=True)

    with tile.TileContext(nc) as tc, tc.tile_pool(name="p", bufs=2) as pool:
        for i in range(NT):
            xt = pool.tile([P, FD], F32, tag="xt")
            nc.sync.dma_start(xt[:], x_ext[i])
            # u' = x*(8/7) + (64/7 - 0.5), fp16 out.  Mostly on ACT; a small
            # share on DVE to balance engine load.
            up = pool.tile([P, FD], F16, tag="up")
            if i % 8 == 7 and False:
                nc.vector.tensor_scalar(
                    up[:], xt[:], C87, UP_BIAS, Alu.mult, Alu.add
                )
            else:
                nc.scalar.activation(up[:], xt[:], Act.Copy, bias=UP_BIAS, scale=C87)
            # kfm = fp16(u' + 1029) = 1029 + floor(u)   (fp16 ulp-1 snap)
            kfm = pool.tile([P, FD], F16, tag="kfm")
            nc.vector.tensor_single_scalar(kfm[:], up[:], MAGIC, Alu.add)
            # kfm2 = k - 0.5   (fp16 exact)
            kfm2 = pool.tile([P, FD], F16, tag="kfm2")
            nc.vector.tensor_single_scalar(kfm2[:], kfm[:], MAGIC2, Alu.add)
            # fr = u' - kfm2 = u - k   (fp16 2x)
            fr = pool.tile([P, FD], F16, tag="fr")
            nc.vector.tensor_tensor(fr[:], up[:], kfm2[:], Alu.subtract)
            # a = silu(t_k), b = silu(t_k + 0.875)   (fp16, free input FMA)
            a = pool.tile([P, FD], F16, tag="a")
            nc.scalar.activation(a[:], kfm[:], Act.Silu, bias=SILU_BIAS_A, scale=0.875)
            b = pool.tile([P, FD], F16, tag="b")
            nc.scalar.activation(b[:], kfm[:], Act.Silu, bias=SILU_BIAS_B, scale=0.875)
            # t = a - b   (fp16 2x)
            t = pool.tile([P, FD], F16, tag="t")
            nc.vector.tensor_tensor(t[:], a[:], b[:], Alu.subtract)
            # g = t*fr = (a-b)*fr   (fp16 2x)
            g = pool.tile([P, FD], F16, tag="g")
            nc.vector.tensor_tensor(g[:], t[:], fr[:], Alu.mult)
            # o = a - g = a + (b-a)*fr   (fp16 2x)
            o = pool.tile([P, FD], F16, tag="o")
            nc.vector.tensor_tensor(o[:], a[:], g[:], Alu.subtract)
            nc.sync.dma_start(o_ext[i], o[:])
    nc.compile()
    return nc


_NC_CACHE = None


def _get_nc():
    global _NC_CACHE
    if _NC_CACHE is None:
        _NC_CACHE = build()
    return _NC_CACHE


def _ensure_ntff_hook():
    """Install the antenv.axon_hooks shim so trace=True works under axon."""
    import sys
    import types

    if "antenv.axon_hooks" not in sys.modules:
        mod = types.ModuleType("antenv.axon_hooks")
        _h = [None]
        mod.set_axon_ntff_profile_hook = lambda h: _h.__setitem__(0, h)
        mod.get_axon_ntff_profile_hook = lambda: _h[0]
        sys.modules["antenv.axon_hooks"] = mod
        import antenv

        antenv.axon_hooks = mod
    import antenv.axon_hooks as ah

    if ah.get_axon_ntff_profile_hook() is None:
        from trn_agent_boot.trn_boot import _ntff_profile_via_ctypes

        h = _ntff_profile_via_ctypes("/opt/axon/libaxon_pjrt.so")
        if h is not None:
            ah.set_axon_ntff_profile_hook(h)
    # avoid cloud artifact uploads in this container
    import concourse.bass_utils as bu

    bu.upload_artifacts = lambda tmpdir: tmpdir


def _run_once(x, trace=False, trace_kwargs=None):
    nc = _get_nc()
    core_ids = list(range(N_CORES))
    in_maps = [{"x": x[i].reshape(NT, P, FD)} for i in range(N_CORES)]
    kwargs = {}
    if trace:
        _ensure_ntff_hook()
        kwargs["trace"] = True
        if trace_kwargs:
            kwargs.update(trace_kwargs)
    res = run_bass_kernel_spmd(nc, in_maps, core_ids, **kwargs)
    out = np.empty((N_CORES, 2048, 4096), dtype=np.float32)
    for i in range(N_CORES):
        out[i] = np.asarray(res.results[i]["out"], dtype=np.float32).reshape(
            2048, 4096
        )
    return out, res.exec_time_ns


def _run(x, trace=False, trace_kwargs=None):
    """x: (8, 2048, 4096) float32. Returns (out, exec_time_ns|None)."""
    x = np.ascontiguousarray(np.asarray(x, dtype=np.float32))
    assert x.shape == (N_CORES, 2048, 4096), x.shape
    # The axon terminal occasionally reports a transient unrecoverable
    # error on the first execution of a freshly loaded NEFF; retry.
    last_exc = None
    for _attempt in range(3):
        try:
            return _run_once(x, trace=trace, trace_kwargs=trace_kwargs)
        except Exception as e:  # noqa: BLE001
            last_exc = e
            import time

            time.sleep(2.0)
    raise last_exc


def kernel(x, seg=None, silu_vals=None, **_unused):
    out, _ = _run(x, trace=False)
    return out


# revision 24
# speedup vs baseline: 1.2181x; 1.0212x over previous
"""ApproxSiLU16_FXP Trainium2 kernel (8 NeuronCores, data-parallel).

The reference computes a 16-segment piecewise-linear fixed-point
approximation of SiLU on a uniform knot grid t_k = -8 + 0.875k
(k = 0..16), with knot values y_k = round(1024*silu(t_k))/1024.
Instead of gathering from the LUT per element, this kernel
reconstructs the same piecewise-linear function analytically:

    u   = x*(8/7) + 64/7          (segment coordinate, in [0,16])
    k   = floor(u)
    fr  = u - k
    out = silu(t_k) + fr*(silu(t_k+0.875) - silu(t_k))

using the ScalarEngine's Silu activation for the knot values.  This
matches the fixed-point reference to ~2e-3 relative error (reference
LUT int rounding + fp16 rounding of the blend), well under the 2e-2
gate.

floor() uses a small-magic fp16 trick: u' = u - 0.5 rounded to fp16,
then kfm = fp16(u' + 1029).  1029 + [-0.5, 16.1] lies inside the fp16
binade [1024, 2048) where the fp16 ulp is exactly 1.0, so the output
rounding (DVE computes fp32 internally, rounds on the write) snaps to
1029 + floor(u).  Both Silus then use the free input FMA:
t_k = 0.875*kfm - 908.375 (exact in fp32).  fr = u' - (kfm - 1029.5)
needs one more fp16 scalar-add and one fp16 subtract, all in fast
16-bit DVE modes.

Engine split per tile (balanced ~183us each at FD=2048):
  ACT : u' (Copy FMA -> fp16, most tiles), a = silu(...), b = silu(...)
  DVE : kfm, kfm2 (fp16 1-scalar add, ~4x), fr, t, g, o (fp16 2x TT)
out = a - (a-b)*fr, stored fp16; the host upcasts to fp32.

Sharding: x is (8, 2048, 4096); core i processes batch row i.
"""

import numpy as np

from concourse import bacc, mybir
import concourse.tile as tile
from concourse.bass_utils import run_bass_kernel_spmd

F32 = mybir.dt.float32
F16 = mybir.dt.float16
Alu = mybir.AluOpType
Act = mybir.ActivationFunctionType

P = 128          # SBUF partitions
FD = 4096        # free dim per tile
NT = 16          # tiles per core shard: 2048*4096 = NT*P*FD
N_CORES = 8

C87 = float(8.0 / 7.0)
UP_BIAS = float(64.0 / 7.0 - 0.5)
MAGIC = 1029.0           # fp16 binade [1024,2048): ulp = 1
MAGIC2 = -1029.5         # kfm2 = k - 0.5
SILU_BIAS_A = -908.375   # 0.875*(k+1029) - 908.375 = 0.875k - 8 = t_k
SILU_BIAS_B = -907.5     # t_k + 0.875


def _reg_const(nc, val):
    t = nc.alloc_sbuf_tensor(f"const-f32-{val}", [128, 1], F32)
    nc.gpsimd.memset(t.ap(), val)
    nc.const_aps.aps[(F32, float(val))] = t.ap()


def build():
    nc = bacc.Bacc()
    _reg_const(nc, SILU_BIAS_A)
    _reg_const(nc, SILU_BIAS_B)
    nc.all_engine_barrier()
    x_ext = nc.declare_dram_parameter("x", [NT, P, FD], F32, isOutput=False)
    o_ext = nc.declare_dram_parameter("out", [NT, P, FD], F16, isOutput=True)

    with tile.TileContext(nc) as tc, tc.tile_pool(name="p", bufs=2) as pool:
        for i in range(NT):
            xt = pool.tile([P, FD], F32, tag="xt")
            nc.sync.dma_start(xt[:], x_ext[i])
            # u' = x*(8/7) + (64/7 - 0.5), fp16 out.  Mostly on ACT; a small
            # share on DVE to balance engine load.
            up = pool.tile([P, FD], F16, tag="up", bufs=3)
            if i % 8 == 7 and False:
                nc.vector.tensor_scalar(
                    up[:], xt[:], C87, UP_BIAS, Alu.mult, Alu.add
                )
            else:
                nc.scalar.activation(up[:], xt[:], Act.Copy, bias=UP_BIAS, scale=C87)
            # kfm = fp16(u' + 1029) = 1029 + floor(u)   (fp16 ulp-1 snap)
            kfm = pool.tile([P, FD], F16, tag="kfm", bufs=3)
            nc.vector.tensor_single_scalar(kfm[:], up[:], MAGIC, Alu.add)
            # kfm2 = k - 0.5   (fp16 exact)
            kfm2 = pool.tile([P, FD], F16, tag="kfm2")
            nc.vector.tensor_single_scalar(kfm2[:], kfm[:], MAGIC2, Alu.add)
            # fr = u' - kfm2 = u - k   (fp16 2x)
            fr = pool.tile([P, FD], F16, tag="fr")
            nc.vector.tensor_tensor(fr[:], up[:], kfm2[:], Alu.subtract)
            # a = silu(t_k), b = silu(t_k + 0.875)   (fp16, free input FMA)
            a = pool.tile([P, FD], F16, tag="a", bufs=3)
            nc.scalar.activation(a[:], kfm[:], Act.Silu, bias=SILU_BIAS_A, scale=0.875)
            b = pool.tile([P, FD], F16, tag="b")
            nc.scalar.activation(b[:], kfm[:], Act.Silu, bias=SILU_BIAS_B, scale=0.875)
            # t = a - b   (fp16 2x)
            t = pool.tile([P, FD], F16, tag="t")
            nc.vector.tensor_tensor(t[:], a[:], b[:], Alu.subtract)
            # g = t*fr = (a-b)*fr   (fp16 2x)
            g = pool.tile([P, FD], F16, tag="g")
            nc.vector.tensor_tensor(g[:], t[:], fr[:], Alu.mult)
            # o = a - g = a + (b-a)*fr   (fp16 2x)
            o = pool.tile([P, FD], F16, tag="o")
            nc.vector.tensor_tensor(o[:], a[:], g[:], Alu.subtract)
            nc.sync.dma_start(o_ext[i], o[:])
    nc.compile()
    return nc


_NC_CACHE = None


def _get_nc():
    global _NC_CACHE
    if _NC_CACHE is None:
        _NC_CACHE = build()
    return _NC_CACHE


def _ensure_ntff_hook():
    """Install the antenv.axon_hooks shim so trace=True works under axon."""
    import sys
    import types

    if "antenv.axon_hooks" not in sys.modules:
        mod = types.ModuleType("antenv.axon_hooks")
        _h = [None]
        mod.set_axon_ntff_profile_hook = lambda h: _h.__setitem__(0, h)
        mod.get_axon_ntff_profile_hook = lambda: _h[0]
        sys.modules["antenv.axon_hooks"] = mod
        import antenv

        antenv.axon_hooks = mod
    import antenv.axon_hooks as ah

    if ah.get_axon_ntff_profile_hook() is None:
        from trn_agent_boot.trn_boot import _ntff_profile_via_ctypes

        h = _ntff_profile_via_ctypes("/opt/axon/libaxon_pjrt.so")
        if h is not None:
            ah.set_axon_ntff_profile_hook(h)
    # avoid cloud artifact uploads in this container
    import concourse.bass_utils as bu

    bu.upload_artifacts = lambda tmpdir: tmpdir


def _run_once(x, trace=False, trace_kwargs=None):
    nc = _get_nc()
    core_ids = list(range(N_CORES))
    in_maps = [{"x": x[i].reshape(NT, P, FD)} for i in range(N_CORES)]
    kwargs = {}
    if trace:
        _ensure_ntff_hook()
        kwargs["trace"] = True
        if trace_kwargs:
            kwargs.update(trace_kwargs)
    res = run_bass_kernel_spmd(nc, in_maps, core_ids, **kwargs)
    out = np.empty((N_CORES, 2048, 4096), dtype=np.float32)
    for i in range(N_CORES):
        out[i] = np.asarray(res.results[i]["out"], dtype=np.float32).reshape(
            2048, 4096
        )
    return out, res.exec_time_ns


def _run(x, trace=False, trace_kwargs=None):
    """x: (8, 2048, 4096) float32. Returns (out, exec_time_ns|None)."""
    x = np.ascontiguousarray(np.asarray(x, dtype=np.float32))
    assert x.shape == (N_CORES, 2048, 4096), x.shape
    # The axon terminal occasionally reports a transient unrecoverable
    # error on the first execution of a freshly loaded NEFF; retry.
    last_exc = None
    for _attempt in range(3):
        try:
            return _run_once(x, trace=trace, trace_kwargs=trace_kwargs)
        except Exception as e:  # noqa: BLE001
            last_exc = e
            import time

            time.sleep(2.0)
    raise last_exc


def kernel(x, seg=None, silu_vals=None, **_unused):
    out, _ = _run(x, trace=False)
    return out
